# revision 1
# baseline (speedup 1.0000x reference)
"""BoxAttention TRN2 kernel — 8-core data-parallel over the window dim.

Per core: 256 windows x 64 tokens x 384 dim, 12 heads, head_dim 32.
Pipeline per 128-token pair-tile (2 windows), all layouts chosen so no
gather and no cross-core communication is needed:

  xT  (c,tok)  <- DMA-transpose (bf16) or PE-transpose (f32) of x
  qT,kT (kout,tok) <- W_qk^T stationary matmuls, rhs = xT
  v   (tok,kv) <- xT-slice stationary matmuls, rhs = W_v^T  (+ ones col)
  S^T (nk,nq)  <- per (window,head) matmuls, tile_position packed
  E^T          <- exp(S^T) * exp(bias)^T            (bias folded via exp)
  AV  (nq, h*33+d) <- stationary E^T, moving v_ext; col 32 = softmax denom
  attn (tok,c) <- AV * 1/denom
  out (tok,o)  <- attn^T stationary, rhs = W_p^T, + proj_b
"""

import os
import sys
import numpy as np

for _p in ("/opt/trn_rl_repo", "/opt/pypackages"):
    if _p not in sys.path and os.path.isdir(_p):
        sys.path.append(_p)

import ml_dtypes  # noqa: E402

DIM, BOX, H = 384, 4, 12
N = BOX ** 3            # 64 tokens per window
HD = DIM // H           # 32
SCALE = HD ** -0.5
B_ = 2048
NCORES = 8
B_PER = B_ // NCORES    # 256 windows per core
TOK = B_PER * N         # 16384 tokens per core
SUPER = 512             # tokens per super-tile (8 windows)
PAIR = 128              # tokens per pair-tile (2 windows)

MODE = os.environ.get("BOXATTN_MODE", "f32")  # "f32" | "bf16"

_cache = {}


def _build(mode, tok_per_core, reps=1):
    import concourse.bass as bass
    import concourse.mybir as mybir
    import concourse.tile as tile
    from concourse import bacc

    f32 = mybir.dt.float32
    dt = mybir.dt.bfloat16 if mode == "bf16" else f32

    nc = bacc.Bacc("TRN2", target_bir_lowering=False, debug=False)

    x_d = nc.dram_tensor("x", [tok_per_core, DIM], dt, kind="ExternalInput").ap()
    wqk_d = nc.dram_tensor("wqkT", [DIM, 768], dt, kind="ExternalInput").ap()
    wv_d = nc.dram_tensor("wvT", [DIM, DIM], dt, kind="ExternalInput").ap()
    wp_d = nc.dram_tensor("wpT", [DIM, DIM], dt, kind="ExternalInput").ap()
    eb_d = nc.dram_tensor("ebT", [PAIR, 1536], dt, kind="ExternalInput").ap()
    pb_d = nc.dram_tensor("pb", [PAIR, DIM], f32, kind="ExternalInput").ap()
    id_d = nc.dram_tensor("ident", [PAIR, PAIR], f32, kind="ExternalInput").ap()
    out_d = nc.dram_tensor("out", [tok_per_core, DIM], f32, kind="ExternalOutput").ap()

    n_super = tok_per_core // SUPER

    with tile.TileContext(nc) as tc:
        with (
            tc.tile_pool(name="consts", bufs=1) as consts,
            tc.tile_pool(name="xn", bufs=3) as xn_pool,
            tc.tile_pool(name="xt", bufs=3) as xt_pool,
            tc.tile_pool(name="qk", bufs=3) as qk_pool,
            tc.tile_pool(name="v", bufs=3) as v_pool,
            tc.tile_pool(name="er", bufs=4) as er_pool,
            tc.tile_pool(name="et", bufs=4) as et_pool,
            tc.tile_pool(name="av", bufs=4) as av_pool,
            tc.tile_pool(name="avt", bufs=4) as avt_pool,
            tc.tile_pool(name="osb", bufs=4) as o_pool,
            tc.tile_pool(name="inv", bufs=4) as inv_pool,
            tc.tile_pool(name="psA", bufs=2, space="PSUM") as psA,
            tc.tile_pool(name="psS", bufs=4, space="PSUM") as psS,
            tc.tile_pool(name="psB", bufs=2, space="PSUM") as psB,
        ):
            wqk = consts.tile([128, 3, 768], dt)
            nc.sync.dma_start(wqk[:], wqk_d.rearrange("(a p) k -> p a k", p=128))
            wv = consts.tile([128, 3, DIM], dt)
            nc.sync.dma_start(wv[:], wv_d.rearrange("(a p) k -> p a k", p=128))
            wp = consts.tile([128, 3, DIM], dt)
            nc.sync.dma_start(wp[:], wp_d.rearrange("(a p) k -> p a k", p=128))
            eb = consts.tile([PAIR, 1536], dt)
            nc.sync.dma_start(eb[:], eb_d)
            pb = consts.tile([PAIR, DIM], f32)
            nc.sync.dma_start(pb[:], pb_d)
            ident = None
            if mode != "bf16":
                ident = consts.tile([PAIR, PAIR], f32)
                nc.sync.dma_start(ident[:], id_d)
            vbufs = []
            for _i in range(3):
                vper = consts.tile([128, H, 33], dt, tag=f"vper{_i}")
                nc.vector.memset(vper[:, :, 32:33], 1.0)
                vbufs.append(vper)

            for sp in range(n_super * reps):
                t0 = (sp % n_super) * SUPER
                # ---- xT [c, tok] for this super-tile ----
                xt = xt_pool.tile([128, 3, SUPER], dt, tag="xt")
                if mode == "bf16":
                    for cc in range(3):
                        nc.sync.dma_start(
                            out=xt[:, cc, :],
                            in_=x_d[t0 : t0 + SUPER, cc * 128 : (cc + 1) * 128],
                            transpose=True,
                        )
                else:
                    xn = xn_pool.tile([128, 4, DIM], f32, tag="xn")
                    nc.sync.dma_start(
                        xn[:], x_d[t0 : t0 + SUPER, :].rearrange("(b p) c -> p b c", p=128)
                    )
                    for cc in range(3):
                        for tb in range(4):
                            tp = psB.tile([128, 128], f32, tag="bp")
                            nc.tensor.transpose(
                                tp[:], xn[:, tb, cc * 128 : (cc + 1) * 128], ident[:]
                            )
                            nc.scalar.copy(xt[:, cc, tb * 128 : (tb + 1) * 128], tp[:])

                # ---- q,k projections (transposed layout) ----
                qkt = qk_pool.tile([128, 6, SUPER], dt, tag="qkt")
                for j in range(6):
                    ps = psA.tile([128, SUPER], f32, tag="psA")
                    for cc in range(3):
                        nc.tensor.matmul(
                            ps[:],
                            lhsT=wqk[:, cc, j * 128 : (j + 1) * 128],
                            rhs=xt[:, cc, :],
                            start=(cc == 0),
                            stop=(cc == 2),
                        )
                    nc.scalar.copy(qkt[:, j, :], ps[:])

                for blk in range(4):
                    tok0 = t0 + blk * PAIR
                    # ---- v (natural layout, interleaved with ones col) ----
                    vps = psA.tile([128, DIM], f32, tag="psA")
                    for cc in range(3):
                        nc.tensor.matmul(
                            vps[:],
                            lhsT=xt[:, cc, blk * 128 : (blk + 1) * 128],
                            rhs=wv[:, cc, :],
                            start=(cc == 0),
                            stop=(cc == 2),
                        )
                    vsb = vbufs[(sp * 4 + blk) % 3]
                    nc.vector.tensor_copy(
                        vsb[:, :, 0:32], vps[:].rearrange("p (h d) -> p h d", d=32)
                    )

                    # ---- S^T per head: one [32,128]x[32,128] matmul over the
                    # whole pair-tile. Cross-window blocks are garbage; the
                    # bias multiply (eb = 0 there) zeroes them, which makes
                    # E^T block-diagonal so AV is one matmul per head too.
                    # One PSUM bank per PE row-group g=h%4 (concurrent
                    # tile_position matmuls must not share a bank).
                    # Bank g must hold exactly the heads of PE row-group g:
                    # concurrent tile_position matmuls from different row
                    # groups must not write the same PSUM bank.
                    sts = []
                    for _g in range(4):
                        st_g = psS.tile([128, 384], f32, tag="s")
                        sts.append(st_g)
                    for h in range(H):
                        g, j = h % 4, h // 4
                        rp = g * 32
                        f0 = blk * 128
                        nc.tensor.matmul(
                            sts[g][:, j * 128 : (j + 1) * 128],
                            lhsT=qkt[rp : rp + 32, 3 + j, f0 : f0 + 128],
                            rhs=qkt[rp : rp + 32, j, f0 : f0 + 128],
                            start=True,
                            stop=True,
                            tile_position=(rp, 0),
                        )
                    er = er_pool.tile([128, 1536], dt, tag="er")
                    for g in range(4):
                        nc.scalar.activation(
                            er[:, g * 384 : (g + 1) * 384],
                            sts[g][:],
                            mybir.ActivationFunctionType.Exp,
                        )
                    et = et_pool.tile([128, 1536], dt, tag="et")
                    nc.vector.tensor_mul(et[:], er[:], eb[:])

                    # ---- AV (+ denominator in col 32 of each head block) ----
                    avp_t = psB.tile([128, 512], f32, tag="bp")
                    avp = avp_t[:, 0 : H * 33].rearrange("p (h d) -> p h d", d=33)
                    for h in range(H):
                        ec = (h % 4) * 384 + (h // 4) * 128
                        nc.tensor.matmul(
                            avp[:, h, :],
                            lhsT=et[:, ec : ec + 128],
                            rhs=vsb[:, h, :],
                            start=True,
                            stop=True,
                        )
                    inv = inv_pool.tile([128, H], f32, tag="inv")
                    nc.vector.reciprocal(inv[:], avp[:, :, 32])
                    avsb = av_pool.tile([128, H, 32], dt, tag="av")
                    nc.vector.tensor_mul(
                        avsb[:],
                        avp[:, :, 0:32],
                        inv[:, :, None].broadcast_to([128, H, 32]),
                    )

                    # ---- attn^T for the output projection ----
                    avt = avt_pool.tile([128, 3, 128], dt, tag="avt")
                    if mode == "bf16":
                        nc.sync.dma_start(
                            out=avt[:],
                            in_=avsb[:].rearrange("p h d -> p (h d)"),
                            transpose=True,
                        )
                    else:
                        for cc in range(3):
                            tp = psB.tile([128, 128], f32, tag="bp")
                            nc.tensor.transpose(
                                tp[:],
                                avsb[:].rearrange("p h d -> p (h d)")[
                                    :, cc * 128 : (cc + 1) * 128
                                ],
                                ident[:],
                            )
                            nc.scalar.copy(avt[:, cc, :], tp[:])

                    # ---- output projection + bias ----
                    ops = psA.tile([128, DIM], f32, tag="psA")
                    for cc in range(3):
                        nc.tensor.matmul(
                            ops[:],
                            lhsT=avt[:, cc, :],
                            rhs=wp[:, cc, :],
                            start=(cc == 0),
                            stop=(cc == 2),
                        )
                    osb = o_pool.tile([128, DIM], f32, tag="osb")
                    nc.vector.tensor_add(osb[:], ops[:], pb[:])
                    nc.sync.dma_start(out_d[tok0 : tok0 + PAIR, :], osb[:])
    nc.compile()
    return nc


def _get_nc(mode, tok_per_core, reps=1):
    key = (mode, tok_per_core, reps)
    if key not in _cache:
        _cache[key] = _build(mode, tok_per_core, reps)
    return _cache[key]


def _host_prep(x, qkv_w, proj_w, proj_b, bias_table, rel_idx, mode, n_cores):
    np_dt = ml_dtypes.bfloat16 if mode == "bf16" else np.float32
    x = np.asarray(x, np.float32)
    qkv_w = np.asarray(qkv_w, np.float32)
    proj_w = np.asarray(proj_w, np.float32)
    proj_b = np.asarray(proj_b, np.float32)
    bias_table = np.asarray(bias_table, np.float32)
    rel_idx = np.asarray(rel_idx)

    wq = qkv_w[0:DIM] * SCALE
    wk = qkv_w[DIM : 2 * DIM]
    wv = qkv_w[2 * DIM :]
    wqkT = np.concatenate([wq, wk], 0).T.copy().astype(np_dt)  # [384, 768]
    wvT = wv.T.copy().astype(np_dt)
    wpT = proj_w.T.copy().astype(np_dt)

    bias = bias_table[rel_idx.reshape(-1)].reshape(N, N, H)  # [nq, nk, h]
    eb1 = np.exp(bias).transpose(1, 2, 0)  # [nk, h, nq]
    ebT = np.zeros((PAIR, H * PAIR), np.float32)  # cross-window blocks stay 0
    for h in range(H):
        ec = (h % 4) * 384 + (h // 4) * 128
        for w in range(2):
            ebT[w * N : (w + 1) * N, ec + w * N : ec + (w + 1) * N] = eb1[:, h, :]
    ebT = ebT.astype(np_dt)  # [128, 1536]
    pb = np.broadcast_to(proj_b, (PAIR, DIM)).copy().astype(np.float32)
    ident = np.eye(PAIR, dtype=np.float32)

    B = x.shape[0]
    bper = B // n_cores
    xs = x.reshape(B * N, DIM).astype(np_dt)
    in_maps = []
    for c in range(n_cores):
        in_maps.append(
            {
                "x": xs[c * bper * N : (c + 1) * bper * N],
                "wqkT": wqkT,
                "wvT": wvT,
                "wpT": wpT,
                "ebT": ebT,
                "pb": pb,
                "ident": ident,
            }
        )
    return in_maps


def kernel(x, qkv_w, proj_w, proj_b, bias_table, rel_idx):
    from concourse.bass_utils import run_bass_kernel_spmd

    x = np.asarray(x)
    B = x.shape[0]
    n_cores = NCORES
    tok_per_core = (B // n_cores) * N
    nc = _get_nc(MODE, tok_per_core)
    in_maps = _host_prep(x, qkv_w, proj_w, proj_b, bias_table, rel_idx, MODE, n_cores)
    res = run_bass_kernel_spmd(nc, in_maps, list(range(n_cores)))
    out = np.concatenate([r["out"] for r in res.results], 0)
    return out.reshape(B, N, DIM).astype(np.float32)



# revision 25
# speedup vs baseline: 278.5454x; 278.5454x over previous
"""BoxAttention TRN2 kernel — 8-core data-parallel over the window dim.

Per core: 256 windows x 64 tokens x 384 dim, 12 heads, head_dim 32.
All bf16 on the PE; f32 accumulation in PSUM. Layouts chosen so no
gather, no device transposes of x, and no cross-core communication:

  xT   (c,tok)   <- loaded directly (x transposed on host, bf16)
  qT,kT (kout,tok) <- W_qk^T stationary matmuls, rhs = xT
  v    (tok,kv)  <- xT-slice stationary matmuls, rhs = W_v^T
  S^T  (nk,nq)   <- per (pair,head) matmuls, tile_position packed
                    4 row-groups wide; + bias added in PSUM via an
                    identity matmul (cross-window blocks get -1e4 so
                    exp() zeroes them -> E^T block-diagonal)
  E^T            <- exp(S^T + bias) on ACT, bf16 into SBUF
  AV   (nq,h*33) <- stationary E^T slices, moving v_ext; col 32 of
                    each head block = softmax denominator (ones col)
  attn (tok,c)   <- AV * 1/denom (DVE)
  attn^T         <- 3x PE transpose + copies
  out  (tok,o)   <- attn^T stationary, rhs = W_p^T, + proj_b

Timing builds (reps>1) wrap the whole pass in a hardware For_i loop so
the NEFF stays small and wall-clock deltas measure pure device time.
"""

import os
import sys
import numpy as np

for _p in ("/opt/trn_rl_repo", "/opt/pypackages"):
    if _p not in sys.path and os.path.isdir(_p):
        sys.path.append(_p)

import ml_dtypes  # noqa: E402

DIM, BOX, H = 384, 4, 12
N = BOX ** 3            # 64 tokens per window
HD = DIM // H           # 32
SCALE = HD ** -0.5
B_ = 2048
NCORES = 8
B_PER = B_ // NCORES    # 256 windows per core
TOK = B_PER * N         # 16384 tokens per core
SUPER = 512             # tokens per super-tile (8 windows)
PAIR = 128              # tokens per pair-tile (2 windows)
NEG = -10000.0          # cross-window mask added to S before exp

MODE = os.environ.get("BOXATTN_MODE", "bf16")
BUFS = os.environ.get("BOXATTN_BUFS", "")  # e.g. "xt=3,qk=2,er=3,av=3,avt=3,osb=2,inv=3,psS=4,psA=2,psB=2"
_B = dict(xt=3, qk=2, er=3, av=3, avt=3, osb=2, inv=3, psS=0, psA=6, psB=2, psQ=0, share=1)
for _kv in BUFS.split(","):
    if "=" in _kv:
        _k, _v = _kv.split("=")
        _B[_k] = int(_v)

_cache = {}


def _build(mode, tok_per_core, reps=1):
    import concourse.bass as cbass
    import concourse.mybir as mybir
    import concourse.tile as tile
    from concourse import bacc

    f32 = mybir.dt.float32
    dt = mybir.dt.bfloat16

    nc = bacc.Bacc("TRN2", target_bir_lowering=False, debug=False)

    x_d = nc.dram_tensor("xT", [DIM, tok_per_core], dt, kind="ExternalInput").ap()
    wqk_d = nc.dram_tensor("wqkT", [DIM, 768], dt, kind="ExternalInput").ap()
    wv_d = nc.dram_tensor("wvT", [DIM, DIM], dt, kind="ExternalInput").ap()
    wp_d = nc.dram_tensor("wpT", [DIM, DIM], dt, kind="ExternalInput").ap()
    eb_d = nc.dram_tensor("ebP", [PAIR, 768], dt, kind="ExternalInput").ap()
    pb_d = nc.dram_tensor("pb", [PAIR, DIM], f32, kind="ExternalInput").ap()
    id_d = nc.dram_tensor("ident", [PAIR, PAIR], dt, kind="ExternalInput").ap()
    out_d = nc.dram_tensor("out", [tok_per_core, DIM], f32, kind="ExternalOutput").ap()

    n_super = tok_per_core // SUPER
    EXP = mybir.ActivationFunctionType.Exp

    with tile.TileContext(nc) as tc:
        with (
            tc.tile_pool(name="consts", bufs=1) as consts,
            tc.tile_pool(name="xt", bufs=_B["xt"]) as xt_pool,
            tc.tile_pool(name="qk", bufs=_B["qk"]) as qk_pool,
            tc.tile_pool(name="er", bufs=_B["er"]) as er_pool,
            tc.tile_pool(name="av", bufs=_B["av"]) as av_pool,
            tc.tile_pool(name="avt", bufs=_B["avt"]) as avt_pool,
            tc.tile_pool(name="osb", bufs=_B["osb"]) as o_pool,
            tc.tile_pool(name="inv", bufs=_B["inv"]) as inv_pool,
            tc.tile_pool(name="psS", bufs=max(_B["psS"], 1), space="PSUM") as psS,
            tc.tile_pool(name="psA", bufs=_B["psA"], space="PSUM") as psA,
            tc.tile_pool(name="psB", bufs=_B["psB"], space="PSUM") as psB,
            tc.tile_pool(name="psQ", bufs=max(_B["psQ"], 1), space="PSUM") as psQ,
        ):
            wqk = consts.tile([128, 3, 768], dt)
            nc.sync.dma_start(wqk[:], wqk_d.rearrange("(a p) k -> p a k", p=128))
            wv = consts.tile([128, 3, DIM], dt)
            nc.sync.dma_start(wv[:], wv_d.rearrange("(a p) k -> p a k", p=128))
            wp = consts.tile([128, 3, DIM], dt)
            nc.sync.dma_start(wp[:], wp_d.rearrange("(a p) k -> p a k", p=128))
            eb = consts.tile([PAIR, 768], dt)
            nc.sync.dma_start(eb[:], eb_d)
            pb = consts.tile([PAIR, DIM], f32)
            nc.sync.dma_start(pb[:], pb_d)
            ident = consts.tile([PAIR, PAIR], dt)
            nc.sync.dma_start(ident[:], id_d)
            NVB = 6
            vbufs = []
            for _i in range(NVB):
                vper = consts.tile([128, H, 33], dt, tag=f"vper{_i}")
                nc.vector.memset(vper[:, :, 32:33], 1.0)
                vbufs.append(vper)

            def super_body(sp):
                t0 = sp * SUPER
                # ---- xT [c, tok] for this super-tile (pre-transposed) ----
                xt = xt_pool.tile([128, 3, SUPER], dt, tag="xt")
                nc.sync.dma_start(
                    xt[:], x_d[:, t0 : t0 + SUPER].rearrange("(a p) t -> p a t", p=128)
                )

                # ---- q,k projections (transposed layout) ----
                qkt = qk_pool.tile([128, 6, SUPER], dt, tag="qkt")
                for j in range(6):
                    ps = (psQ if _B["psQ"] else psA).tile([128, SUPER], f32, tag="q" if _B["psQ"] else "a")
                    for cc in range(3):
                        nc.tensor.matmul(
                            ps[:],
                            lhsT=wqk[:, cc, j * 128 : (j + 1) * 128],
                            rhs=xt[:, cc, :],
                            start=(cc == 0),
                            stop=(cc == 2),
                        )
                    if j % 2 == 0:
                        nc.vector.tensor_copy(qkt[:, j, :], ps[:])
                    else:
                        nc.scalar.copy(qkt[:, j, :], ps[:])

                osb_s = o_pool.tile([128, 4, DIM], f32, tag="osb")
                vsbs = []
                for blk in range(4):
                    f0 = blk * PAIR
                    # ---- v (natural layout, ones col for denominator) ----
                    vps = psA.tile([128, DIM], f32, tag="a")
                    for cc in range(3):
                        nc.tensor.matmul(
                            vps[:],
                            lhsT=xt[:, cc, f0 : f0 + 128],
                            rhs=wv[:, cc, :],
                            start=(cc == 0),
                            stop=(cc == 2),
                        )
                    vsb = vbufs[(sp * 4 + blk) % NVB]
                    nc.vector.tensor_copy(
                        vsb[:, :, 0:32], vps[:].rearrange("p (h d) -> p h d", d=32)
                    )
                    vsbs.append(vsb)

                for blk in range(4):
                    f0 = blk * PAIR
                    vsb = vsbs[blk]
                    # ---- S^T packed by (head, window), one PSUM bank per PE
                    # row group (heads g, g+4, g+8 -> bank g): all matmuls
                    # into a bank come from the same row group, so they
                    # serialize in the array (two concurrent matmuls from
                    # different row groups must not write the same partitions
                    # of one bank). Within a bank, head i=h//4 owns a 64-col
                    # chunk; window w lives on partition half w*64. Bias
                    # lands first via an identity matmul (start=True), the
                    # 6 per-(head,window) S matmuls accumulate onto it.
                    # Everything in the bank is useful - no cross-window
                    # garbage is ever computed.
                    sts = []
                    prev = []
                    spool = psA if _B.get("share") else psS
                    for _g in range(4):
                        st_g = spool.tile([128, 192], f32, tag="a" if _B.get("share") else "s")
                        sts.append(st_g)
                    for g in range(4):
                        bmm = nc.tensor.matmul(
                            sts[g][:],
                            lhsT=ident[:],
                            rhs=eb[:, g * 192 : (g + 1) * 192],
                            start=True,
                            stop=False,
                            skip_group_check=True,
                        )
                        prev.append(bmm)
                    for h in range(H):
                        g, i = h % 4, h // 4
                        rp = g * 32
                        j = h // 4
                        for w in range(2):
                            mm = nc.tensor.matmul(
                                sts[g][w * 64 : (w + 1) * 64, i * 64 : (i + 1) * 64],
                                lhsT=qkt[rp : rp + 32, 3 + j, f0 + w * 64 : f0 + (w + 1) * 64],
                                rhs=qkt[rp : rp + 32, j, f0 + w * 64 : f0 + (w + 1) * 64],
                                start=False,
                                stop=(i == 2 and w == 1),
                                tile_position=(rp, w * 64),
                                skip_group_check=True,
                            )
                            # accumulation-group order: bias first, stop last
                            cbass._add_dep_helper(
                                mm.ins, prev[g].ins, sync=False, reason="psum group order"
                            )
                            prev[g] = mm
                    er = er_pool.tile([128, 768], dt, tag="er")
                    for g in range(4):
                        nc.scalar.activation(er[:, g * 192 : (g + 1) * 192], sts[g][:], EXP)

                    # ---- AV (+ denominator in col 32 of each head block) ----
                    avp_t = psA.tile([128, 512], f32, tag="a")
                    avp = avp_t[:, 0 : H * 33].rearrange("p (h d) -> p h d", d=33)
                    for h in range(H):
                        g, i = h % 4, h // 4
                        ec = g * 192 + i * 64
                        for w in range(2):
                            nc.tensor.matmul(
                                avp[w * 64 : (w + 1) * 64, h, :],
                                lhsT=er[w * 64 : (w + 1) * 64, ec : ec + 64],
                                rhs=vsb[w * 64 : (w + 1) * 64, h, :],
                                start=True,
                                stop=True,
                                tile_position=(w * 64, w * 64),
                            )
                    inv = inv_pool.tile([128, H], f32, tag="inv")
                    nc.vector.reciprocal(inv[:], avp[:, :, 32])
                    avsb = av_pool.tile([128, H, 32], dt, tag="av")
                    nc.vector.tensor_mul(
                        avsb[:],
                        avp[:, :, 0:32],
                        inv[:, :, None].broadcast_to([128, H, 32]),
                    )

                    # ---- attn^T for the output projection: 3 PE transposes
                    # into one bf16 PSUM tile (768B, single bank), one drain
                    avt = avt_pool.tile([128, 3, 128], dt, tag="avt")
                    av_flat = avsb[:].rearrange("p h d -> p (h d)")
                    tp = psB.tile([128, 3, 128], dt, tag="b")
                    for cc in range(3):
                        nc.tensor.transpose(
                            tp[:, cc, :], av_flat[:, cc * 128 : (cc + 1) * 128], ident[:]
                        )
                    nc.scalar.copy(avt[:], tp[:])

                    # ---- output projection + bias ----
                    ops = psB.tile([128, DIM], f32, tag="b")
                    for cc in range(3):
                        nc.tensor.matmul(
                            ops[:],
                            lhsT=avt[:, cc, :],
                            rhs=wp[:, cc, :],
                            start=(cc == 0),
                            stop=(cc == 2),
                        )
                    nc.vector.tensor_add(osb_s[:, blk, :], ops[:], pb[:])

                nc.sync.dma_start(
                    out_d[t0 : t0 + SUPER, :].rearrange("(b p) c -> p b c", p=128),
                    osb_s[:],
                )

            def full_pass():
                for sp in range(n_super):
                    super_body(sp)

            if reps > 1:
                with tc.For_i(0, reps, 1):
                    full_pass()
            else:
                full_pass()
    nc.compile()
    return nc


def _get_nc(mode, tok_per_core, reps=1):
    key = (mode, tok_per_core, reps)
    if key not in _cache:
        _cache[key] = _build(mode, tok_per_core, reps)
    return _cache[key]


def _host_prep(x, qkv_w, proj_w, proj_b, bias_table, rel_idx, mode, n_cores):
    np_dt = ml_dtypes.bfloat16
    x = np.asarray(x, np.float32)
    qkv_w = np.asarray(qkv_w, np.float32)
    proj_w = np.asarray(proj_w, np.float32)
    proj_b = np.asarray(proj_b, np.float32)
    bias_table = np.asarray(bias_table, np.float32)
    rel_idx = np.asarray(rel_idx)

    wq = qkv_w[0:DIM] * SCALE
    wk = qkv_w[DIM : 2 * DIM]
    wv = qkv_w[2 * DIM :]
    wqkT = np.concatenate([wq, wk], 0).T.copy().astype(np_dt)  # [384, 768]
    wvT = wv.T.copy().astype(np_dt)
    wpT = proj_w.T.copy().astype(np_dt)

    bias = bias_table[rel_idx.reshape(-1)].reshape(N, N, H)  # [nq, nk, h]
    # (head, window)-packed S layout: bank g=h%4 cols [g*192,(g+1)*192)
    # holds heads g, g+4, g+8 in 64-col chunks (i=h//4); window w on
    # partition half w*64. Both halves of a chunk carry the same bias.
    ebP = np.zeros((PAIR, 768), np.float32)
    for h in range(H):
        g, i = h % 4, h // 4
        ec = g * 192 + i * 64
        bT = bias[:, :, h].T  # [nk, nq]
        for w in range(2):
            ebP[w * N : (w + 1) * N, ec : ec + N] = bT
    ebP = ebP.astype(np_dt)  # [128, 768]
    pb = np.broadcast_to(proj_b, (PAIR, DIM)).copy().astype(np.float32)
    ident = np.eye(PAIR, dtype=np.float32).astype(np_dt)

    B = x.shape[0]
    bper = B // n_cores
    xs = x.reshape(B * N, DIM)
    in_maps = []
    for c in range(n_cores):
        xT = np.ascontiguousarray(
            xs[c * bper * N : (c + 1) * bper * N].T
        ).astype(np_dt)  # [384, TOK]
        in_maps.append(
            {
                "xT": xT,
                "wqkT": wqkT,
                "wvT": wvT,
                "wpT": wpT,
                "ebP": ebP,
                "pb": pb,
                "ident": ident,
            }
        )
    return in_maps


def kernel(x, qkv_w, proj_w, proj_b, bias_table, rel_idx):
    from concourse.bass_utils import run_bass_kernel_spmd

    x = np.asarray(x)
    B = x.shape[0]
    n_cores = NCORES
    tok_per_core = (B // n_cores) * N
    nc = _get_nc(MODE, tok_per_core)
    in_maps = _host_prep(x, qkv_w, proj_w, proj_b, bias_table, rel_idx, MODE, n_cores)
    res = run_bass_kernel_spmd(nc, in_maps, list(range(n_cores)))
    out = np.concatenate([r["out"] for r in res.results], 0)
    return out.reshape(B, N, DIM).astype(np.float32)


# revision 27
# speedup vs baseline: 287.1792x; 1.0310x over previous
"""BoxAttention TRN2 kernel — 8-core data-parallel over the window dim.

Per core: 256 windows x 64 tokens x 384 dim, 12 heads, head_dim 32.
All bf16 on the PE; f32 accumulation in PSUM. Layouts chosen so no
gather, no device transposes of x, and no cross-core communication:

  xT   (c,tok)   <- loaded directly (x transposed on host, bf16)
  qT,kT (kout,tok) <- W_qk^T stationary matmuls, rhs = xT
  v    (tok,kv)  <- xT-slice stationary matmuls, rhs = W_v^T
  S^T  (nk,nq)   <- per (pair,head) matmuls, tile_position packed
                    4 row-groups wide; + bias added in PSUM via an
                    identity matmul (cross-window blocks get -1e4 so
                    exp() zeroes them -> E^T block-diagonal)
  E^T            <- exp(S^T + bias) on ACT, bf16 into SBUF
  AV   (nq,h*33) <- stationary E^T slices, moving v_ext; col 32 of
                    each head block = softmax denominator (ones col)
  attn (tok,c)   <- AV * 1/denom (DVE)
  attn^T         <- 3x PE transpose + copies
  out  (tok,o)   <- attn^T stationary, rhs = W_p^T, + proj_b

Timing builds (reps>1) wrap the whole pass in a hardware For_i loop so
the NEFF stays small and wall-clock deltas measure pure device time.
"""

import os
import sys
import numpy as np

for _p in ("/opt/trn_rl_repo", "/opt/pypackages"):
    if _p not in sys.path and os.path.isdir(_p):
        sys.path.append(_p)

import ml_dtypes  # noqa: E402

DIM, BOX, H = 384, 4, 12
N = BOX ** 3            # 64 tokens per window
HD = DIM // H           # 32
SCALE = HD ** -0.5
B_ = 2048
NCORES = 8
B_PER = B_ // NCORES    # 256 windows per core
TOK = B_PER * N         # 16384 tokens per core
SUPER = 512             # tokens per super-tile (8 windows)
PAIR = 128              # tokens per pair-tile (2 windows)
NEG = -10000.0          # cross-window mask added to S before exp

MODE = os.environ.get("BOXATTN_MODE", "bf16")
BUFS = os.environ.get("BOXATTN_BUFS", "")  # e.g. "xt=3,qk=2,er=3,av=3,avt=3,osb=2,inv=3,psS=4,psA=2,psB=2"
_B = dict(xt=3, qk=2, er=3, av=3, avt=3, osb=2, inv=3, psS=0, psA=6, psB=2, psQ=0, share=1)
for _kv in BUFS.split(","):
    if "=" in _kv:
        _k, _v = _kv.split("=")
        _B[_k] = int(_v)

_cache = {}


def _build(mode, tok_per_core, reps=1):
    import concourse.bass as cbass
    import concourse.mybir as mybir
    import concourse.tile as tile
    from concourse import bacc

    f32 = mybir.dt.float32
    dt = mybir.dt.bfloat16

    nc = bacc.Bacc("TRN2", target_bir_lowering=False, debug=False)

    x_d = nc.dram_tensor("xT", [DIM, tok_per_core], dt, kind="ExternalInput").ap()
    wqk_d = nc.dram_tensor("wqkT", [DIM, 768], dt, kind="ExternalInput").ap()
    wv_d = nc.dram_tensor("wvT", [DIM, DIM], dt, kind="ExternalInput").ap()
    wp_d = nc.dram_tensor("wpT", [DIM, DIM], dt, kind="ExternalInput").ap()
    eb_d = nc.dram_tensor("ebP", [PAIR, 768], dt, kind="ExternalInput").ap()
    pb_d = nc.dram_tensor("pb", [PAIR, DIM], f32, kind="ExternalInput").ap()
    id_d = nc.dram_tensor("ident", [PAIR, PAIR], dt, kind="ExternalInput").ap()
    out_d = nc.dram_tensor("out", [tok_per_core, DIM], f32, kind="ExternalOutput").ap()

    n_super = tok_per_core // SUPER
    EXP = mybir.ActivationFunctionType.Exp

    with tile.TileContext(nc) as tc:
        with (
            tc.tile_pool(name="consts", bufs=1) as consts,
            tc.tile_pool(name="xt", bufs=_B["xt"]) as xt_pool,
            tc.tile_pool(name="qk", bufs=_B["qk"]) as qk_pool,
            tc.tile_pool(name="er", bufs=_B["er"]) as er_pool,
            tc.tile_pool(name="et", bufs=_B["er"]) as et_pool,
            tc.tile_pool(name="av", bufs=_B["av"]) as av_pool,
            tc.tile_pool(name="avt", bufs=_B["avt"]) as avt_pool,
            tc.tile_pool(name="osb", bufs=_B["osb"]) as o_pool,
            tc.tile_pool(name="inv", bufs=_B["inv"]) as inv_pool,
            tc.tile_pool(name="psS", bufs=max(_B["psS"], 1), space="PSUM") as psS,
            tc.tile_pool(name="psA", bufs=_B["psA"], space="PSUM") as psA,
            tc.tile_pool(name="psB", bufs=_B["psB"], space="PSUM") as psB,
            tc.tile_pool(name="psQ", bufs=max(_B["psQ"], 1), space="PSUM") as psQ,
        ):
            wqk = consts.tile([128, 3, 768], dt)
            nc.sync.dma_start(wqk[:], wqk_d.rearrange("(a p) k -> p a k", p=128))
            wv = consts.tile([128, 3, DIM], dt)
            nc.sync.dma_start(wv[:], wv_d.rearrange("(a p) k -> p a k", p=128))
            wp = consts.tile([128, 3, DIM], dt)
            nc.sync.dma_start(wp[:], wp_d.rearrange("(a p) k -> p a k", p=128))
            eb = consts.tile([PAIR, 768], dt)
            nc.sync.dma_start(eb[:], eb_d)
            pb = consts.tile([PAIR, DIM], f32)
            nc.sync.dma_start(pb[:], pb_d)
            ident = consts.tile([PAIR, PAIR], dt)
            nc.sync.dma_start(ident[:], id_d)
            NVB = 6
            vbufs = []
            for _i in range(NVB):
                vper = consts.tile([128, H, 33], dt, tag=f"vper{_i}")
                nc.vector.memset(vper[:, :, 32:33], 1.0)
                vbufs.append(vper)

            def super_body(sp):
                t0 = sp * SUPER
                # ---- xT [c, tok] for this super-tile (pre-transposed) ----
                xt = xt_pool.tile([128, 3, SUPER], dt, tag="xt")
                nc.sync.dma_start(
                    xt[:], x_d[:, t0 : t0 + SUPER].rearrange("(a p) t -> p a t", p=128)
                )

                # ---- q,k projections (transposed layout) ----
                qkt = qk_pool.tile([128, 6, SUPER], dt, tag="qkt")
                for j in range(6):
                    ps = (psQ if _B["psQ"] else psA).tile([128, SUPER], f32, tag="q" if _B["psQ"] else "a")
                    for cc in range(3):
                        nc.tensor.matmul(
                            ps[:],
                            lhsT=wqk[:, cc, j * 128 : (j + 1) * 128],
                            rhs=xt[:, cc, :],
                            start=(cc == 0),
                            stop=(cc == 2),
                        )
                    if j % 2 == 0:
                        nc.vector.tensor_copy(qkt[:, j, :], ps[:])
                    else:
                        nc.scalar.copy(qkt[:, j, :], ps[:])

                osb_s = o_pool.tile([128, 4, DIM], f32, tag="osb")
                vsbs = []
                for blk in range(4):
                    f0 = blk * PAIR
                    # ---- v (natural layout, ones col for denominator) ----
                    vps = psA.tile([128, DIM], f32, tag="a")
                    for cc in range(3):
                        nc.tensor.matmul(
                            vps[:],
                            lhsT=xt[:, cc, f0 : f0 + 128],
                            rhs=wv[:, cc, :],
                            start=(cc == 0),
                            stop=(cc == 2),
                        )
                    vsb = vbufs[(sp * 4 + blk) % NVB]
                    nc.vector.tensor_copy(
                        vsb[:, :, 0:32], vps[:].rearrange("p (h d) -> p h d", d=32)
                    )
                    vsbs.append(vsb)

                for blk in range(4):
                    f0 = blk * PAIR
                    vsb = vsbs[blk]
                    # ---- S^T packed by (head, window), one PSUM bank per PE
                    # row group (heads g, g+4, g+8 -> bank g): all matmuls
                    # into a bank come from the same row group, so they
                    # serialize in the array (two concurrent matmuls from
                    # different row groups must not write the same partitions
                    # of one bank). Within a bank, head i=h//4 owns a 64-col
                    # chunk; window w lives on partition half w*64. All
                    # singleton accumulation groups; w-outer order so
                    # consecutive LDWEIGHTS hit different row groups and
                    # overlap in-flight matmuls. Everything in each bank is
                    # useful - no cross-window garbage is ever computed.
                    sts = []
                    spool = psA if _B.get("share") else psS
                    for _g in range(4):
                        st_g = spool.tile([128, 192], f32, tag="a" if _B.get("share") else "s")
                        sts.append(st_g)
                    for w in range(2):
                        for h in range(H):
                            g, i = h % 4, h // 4
                            rp = g * 32
                            j = h // 4
                            nc.tensor.matmul(
                                sts[g][w * 64 : (w + 1) * 64, i * 64 : (i + 1) * 64],
                                lhsT=qkt[rp : rp + 32, 3 + j, f0 + w * 64 : f0 + (w + 1) * 64],
                                rhs=qkt[rp : rp + 32, j, f0 + w * 64 : f0 + (w + 1) * 64],
                                start=True,
                                stop=True,
                                tile_position=(rp, w * 64),
                            )
                    # E^T = exp(S) * exp(bias), bias multiplicative on DVE
                    # (bf16 2x mode); eb holds exp(bias) from the host.
                    er = er_pool.tile([128, 768], dt, tag="er")
                    et = et_pool.tile([128, 768], dt, tag="et")
                    for g in range(4):
                        nc.scalar.activation(er[:, g * 192 : (g + 1) * 192], sts[g][:], EXP)
                    for gg in range(4):
                        nc.vector.tensor_mul(
                            et[:, gg * 192 : (gg + 1) * 192],
                            er[:, gg * 192 : (gg + 1) * 192],
                            eb[:, gg * 192 : (gg + 1) * 192],
                        )

                    # ---- AV (+ denominator in col 32 of each head block) ----
                    avp_t = psA.tile([128, 512], f32, tag="a")
                    avp = avp_t[:, 0 : H * 33].rearrange("p (h d) -> p h d", d=33)
                    for h in range(H):
                        g, i = h % 4, h // 4
                        ec = g * 192 + i * 64
                        for w in range(2):
                            nc.tensor.matmul(
                                avp[w * 64 : (w + 1) * 64, h, :],
                                lhsT=et[w * 64 : (w + 1) * 64, ec : ec + 64],
                                rhs=vsb[w * 64 : (w + 1) * 64, h, :],
                                start=True,
                                stop=True,
                                tile_position=(w * 64, w * 64),
                            )
                    inv = inv_pool.tile([128, H], f32, tag="inv")
                    nc.vector.reciprocal(inv[:], avp[:, :, 32])
                    avsb = av_pool.tile([128, H, 32], dt, tag="av")
                    nc.vector.tensor_mul(
                        avsb[:],
                        avp[:, :, 0:32],
                        inv[:, :, None].broadcast_to([128, H, 32]),
                    )

                    # ---- attn^T for the output projection: 3 PE transposes
                    # into one bf16 PSUM tile (768B, single bank), one drain
                    avt = avt_pool.tile([128, 3, 128], dt, tag="avt")
                    av_flat = avsb[:].rearrange("p h d -> p (h d)")
                    tp = psB.tile([128, 3, 128], dt, tag="b")
                    for cc in range(3):
                        nc.tensor.transpose(
                            tp[:, cc, :], av_flat[:, cc * 128 : (cc + 1) * 128], ident[:]
                        )
                    nc.scalar.copy(avt[:], tp[:])

                    # ---- output projection + bias ----
                    ops = psB.tile([128, DIM], f32, tag="b")
                    for cc in range(3):
                        nc.tensor.matmul(
                            ops[:],
                            lhsT=avt[:, cc, :],
                            rhs=wp[:, cc, :],
                            start=(cc == 0),
                            stop=(cc == 2),
                        )
                    nc.vector.tensor_add(osb_s[:, blk, :], ops[:], pb[:])

                nc.sync.dma_start(
                    out_d[t0 : t0 + SUPER, :].rearrange("(b p) c -> p b c", p=128),
                    osb_s[:],
                )

            def full_pass():
                for sp in range(n_super):
                    super_body(sp)

            if reps > 1:
                with tc.For_i(0, reps, 1):
                    full_pass()
            else:
                full_pass()
    nc.compile()
    return nc


def _get_nc(mode, tok_per_core, reps=1):
    key = (mode, tok_per_core, reps)
    if key not in _cache:
        _cache[key] = _build(mode, tok_per_core, reps)
    return _cache[key]


def _host_prep(x, qkv_w, proj_w, proj_b, bias_table, rel_idx, mode, n_cores):
    np_dt = ml_dtypes.bfloat16
    x = np.asarray(x, np.float32)
    qkv_w = np.asarray(qkv_w, np.float32)
    proj_w = np.asarray(proj_w, np.float32)
    proj_b = np.asarray(proj_b, np.float32)
    bias_table = np.asarray(bias_table, np.float32)
    rel_idx = np.asarray(rel_idx)

    wq = qkv_w[0:DIM] * SCALE
    wk = qkv_w[DIM : 2 * DIM]
    wv = qkv_w[2 * DIM :]
    wqkT = np.concatenate([wq, wk], 0).T.copy().astype(np_dt)  # [384, 768]
    wvT = wv.T.copy().astype(np_dt)
    wpT = proj_w.T.copy().astype(np_dt)

    bias = bias_table[rel_idx.reshape(-1)].reshape(N, N, H)  # [nq, nk, h]
    # (head, window)-packed S layout: bank g=h%4 cols [g*192,(g+1)*192)
    # holds heads g, g+4, g+8 in 64-col chunks (i=h//4); window w on
    # partition half w*64. Both halves of a chunk carry the same bias.
    ebP = np.zeros((PAIR, 768), np.float32)
    for h in range(H):
        g, i = h % 4, h // 4
        ec = g * 192 + i * 64
        bT = bias[:, :, h].T  # [nk, nq]
        for w in range(2):
            ebP[w * N : (w + 1) * N, ec : ec + N] = bT
    ebP = np.exp(ebP).astype(np_dt)  # exp(bias), [128, 768]
    pb = np.broadcast_to(proj_b, (PAIR, DIM)).copy().astype(np.float32)
    ident = np.eye(PAIR, dtype=np.float32).astype(np_dt)

    B = x.shape[0]
    bper = B // n_cores
    xs = x.reshape(B * N, DIM)
    in_maps = []
    for c in range(n_cores):
        xT = np.ascontiguousarray(
            xs[c * bper * N : (c + 1) * bper * N].T
        ).astype(np_dt)  # [384, TOK]
        in_maps.append(
            {
                "xT": xT,
                "wqkT": wqkT,
                "wvT": wvT,
                "wpT": wpT,
                "ebP": ebP,
                "pb": pb,
                "ident": ident,
            }
        )
    return in_maps


def kernel(x, qkv_w, proj_w, proj_b, bias_table, rel_idx):
    from concourse.bass_utils import run_bass_kernel_spmd

    x = np.asarray(x)
    B = x.shape[0]
    n_cores = NCORES
    tok_per_core = (B // n_cores) * N
    nc = _get_nc(MODE, tok_per_core)
    in_maps = _host_prep(x, qkv_w, proj_w, proj_b, bias_table, rel_idx, MODE, n_cores)
    res = run_bass_kernel_spmd(nc, in_maps, list(range(n_cores)))
    out = np.concatenate([r["out"] for r in res.results], 0)
    return out.reshape(B, N, DIM).astype(np.float32)


# revision 33
# speedup vs baseline: 375.0264x; 1.3059x over previous
"""BoxAttention TRN2 kernel — 8-core data-parallel over the window dim.

Per core: 256 windows x 64 tokens x 384 dim, 12 heads, head_dim 32.
All bf16 on the PE; f32 accumulation in PSUM. Layouts chosen so no
gather, no device transposes of x, and no cross-core communication:

  xT   (c,tok)   <- loaded directly (x transposed on host, bf16)
  qT,kT (kout,tok) <- W_qk^T stationary matmuls, rhs = xT
  v    (tok,kv)  <- xT-slice stationary matmuls, rhs = W_v^T
  S^T            <- 24 per-(head,window) [32c x 64 x 64] matmuls,
                    tile_position packed: one PSUM bank per PE row
                    group (heads h%4==g -> bank g), window w on
                    partition half w*64 - every element computed is
                    useful, and same-bank matmuls share a row group
                    (concurrent cross-row-group writes to one bank's
                    partitions crash the device)
  E^T            <- exp(S^T) on ACT, then * exp(bias) on DVE (bf16)
  AV   (nq,h*33) <- stationary E^T slices, moving v_ext; col 32 of
                    each head block = softmax denominator (ones col)
  attn (tok,c)   <- AV * 1/denom (DVE)
  attn^T         <- 3x PE transpose into one bf16 PSUM tile, 1 drain
  out  (tok,o)   <- attn^T stationary, rhs = W_p^T, + proj_b

Timing builds (reps>1) wrap the whole pass in a hardware For_i loop so
the NEFF stays small and wall-clock deltas measure pure device time.
"""

import os
import sys
import numpy as np

for _p in ("/opt/trn_rl_repo", "/opt/pypackages"):
    if _p not in sys.path and os.path.isdir(_p):
        sys.path.append(_p)

import ml_dtypes  # noqa: E402

DIM, BOX, H = 384, 4, 12
N = BOX ** 3            # 64 tokens per window
HD = DIM // H           # 32
SCALE = HD ** -0.5
B_ = 2048
NCORES = 8
B_PER = B_ // NCORES    # 256 windows per core
TOK = B_PER * N         # 16384 tokens per core
SUPER = int(os.environ.get("BOXATTN_SUPER", "512"))  # tokens per super-tile
PAIR = 128              # tokens per pair-tile (2 windows)

MODE = os.environ.get("BOXATTN_MODE", "bf16")
BUFS = os.environ.get("BOXATTN_BUFS", "")  # e.g. "xt=3,qk=2,er=3,av=3,avt=3,osb=2,inv=3,psS=4,psA=2,psB=2"
_B = dict(xt=3, qk=2, er=3, av=5, avt=3, osb=2, inv=5, psS=0, psA=5, psB=3, psQ=0, share=1)
for _kv in BUFS.split(","):
    if "=" in _kv:
        _k, _v = _kv.split("=")
        _B[_k] = int(_v)

_cache = {}


def _build(mode, tok_per_core, reps=1):
    import concourse.bass as cbass
    import concourse.mybir as mybir
    import concourse.tile as tile
    from concourse import bacc

    f32 = mybir.dt.float32
    dt = mybir.dt.bfloat16

    nc = bacc.Bacc("TRN2", target_bir_lowering=False, debug=False)

    x_d = nc.dram_tensor("xT", [DIM, tok_per_core], dt, kind="ExternalInput").ap()
    wqk_d = nc.dram_tensor("wqkT", [DIM, 768], dt, kind="ExternalInput").ap()
    wv_d = nc.dram_tensor("wvT", [DIM, DIM], dt, kind="ExternalInput").ap()
    wp_d = nc.dram_tensor("wpT", [DIM, DIM], dt, kind="ExternalInput").ap()
    eb_d = nc.dram_tensor("ebP", [PAIR, 768], dt, kind="ExternalInput").ap()
    pb_d = nc.dram_tensor("pb", [PAIR, DIM], f32, kind="ExternalInput").ap()
    id_d = nc.dram_tensor("ident", [PAIR, PAIR], dt, kind="ExternalInput").ap()
    out_d = nc.dram_tensor("out", [tok_per_core, DIM], f32, kind="ExternalOutput").ap()

    n_super = tok_per_core // SUPER
    EXP = mybir.ActivationFunctionType.Exp

    with tile.TileContext(nc) as tc:
        with (
            tc.tile_pool(name="consts", bufs=1) as consts,
            tc.tile_pool(name="xt", bufs=_B["xt"]) as xt_pool,
            tc.tile_pool(name="qk", bufs=_B["qk"]) as qk_pool,
            tc.tile_pool(name="er", bufs=_B["er"]) as er_pool,
            tc.tile_pool(name="et", bufs=_B["er"]) as et_pool,
            tc.tile_pool(name="av", bufs=_B["av"]) as av_pool,
            tc.tile_pool(name="avt", bufs=_B["avt"]) as avt_pool,
            tc.tile_pool(name="osb", bufs=_B["osb"]) as o_pool,
            tc.tile_pool(name="inv", bufs=_B["inv"]) as inv_pool,
            tc.tile_pool(name="psS", bufs=max(_B["psS"], 1), space="PSUM") as psS,
            tc.tile_pool(name="psA", bufs=_B["psA"], space="PSUM") as psA,
            tc.tile_pool(name="psB", bufs=_B["psB"], space="PSUM") as psB,
            tc.tile_pool(name="psQ", bufs=max(_B["psQ"], 1), space="PSUM") as psQ,
        ):
            wqk = consts.tile([128, 3, 768], dt)
            nc.sync.dma_start(wqk[:], wqk_d.rearrange("(a p) k -> p a k", p=128))
            wv = consts.tile([128, 3, DIM], dt)
            nc.sync.dma_start(wv[:], wv_d.rearrange("(a p) k -> p a k", p=128))
            wp = consts.tile([128, 3, DIM], dt)
            nc.sync.dma_start(wp[:], wp_d.rearrange("(a p) k -> p a k", p=128))
            eb = consts.tile([PAIR, 768], dt)
            nc.sync.dma_start(eb[:], eb_d)
            pb = consts.tile([PAIR, DIM], f32)
            nc.sync.dma_start(pb[:], pb_d)
            ident = consts.tile([PAIR, PAIR], dt)
            nc.sync.dma_start(ident[:], id_d)
            NVB = 6
            vbufs = []
            for _i in range(NVB):
                vper = consts.tile([128, H, 33], dt, tag=f"vper{_i}")
                nc.vector.memset(vper[:, :, 32:33], 1.0)
                vbufs.append(vper)

            def super_body(sp):
                t0 = sp * SUPER
                # ---- xT [c, tok] for this super-tile (pre-transposed) ----
                xt = xt_pool.tile([128, 3, SUPER], dt, tag="xt")
                nc.sync.dma_start(
                    xt[:], x_d[:, t0 : t0 + SUPER].rearrange("(a p) t -> p a t", p=128)
                )

                # ---- q,k projections (transposed layout) ----
                qkt = qk_pool.tile([128, 6, SUPER], dt, tag="qkt")
                for j in range(6):
                    for hh in range(SUPER // 512):
                        c0 = hh * 512
                        ps = (psQ if _B["psQ"] else psA).tile([128, 512], f32, tag="q" if _B["psQ"] else "a")
                        for cc in range(3):
                            nc.tensor.matmul(
                                ps[:],
                                lhsT=wqk[:, cc, j * 128 : (j + 1) * 128],
                                rhs=xt[:, cc, c0 : c0 + 512],
                                start=(cc == 0),
                                stop=(cc == 2),
                            )
                        if j % 2 == 0:
                            nc.vector.tensor_copy(qkt[:, j, c0 : c0 + 512], ps[:])
                        else:
                            nc.scalar.copy(qkt[:, j, c0 : c0 + 512], ps[:])

                NPAIR = SUPER // PAIR
                osb_s = o_pool.tile([128, NPAIR, DIM], f32, tag="osb")
                vsbs = []
                for blk in range(NPAIR):
                    f0 = blk * PAIR
                    # ---- v (natural layout, ones col for denominator) ----
                    vps = psA.tile([128, DIM], f32, tag="a")
                    for cc in range(3):
                        nc.tensor.matmul(
                            vps[:],
                            lhsT=xt[:, cc, f0 : f0 + 128],
                            rhs=wv[:, cc, :],
                            start=(cc == 0),
                            stop=(cc == 2),
                        )
                    vsb = vbufs[(sp * 4 + blk) % NVB]
                    nc.vector.tensor_copy(
                        vsb[:, :, 0:32], vps[:].rearrange("p (h d) -> p h d", d=32)
                    )
                    vsbs.append(vsb)

                avsbs = []
                for blk in range(NPAIR):
                    f0 = blk * PAIR
                    vsb = vsbs[blk]
                    sts = []
                    spool = psA if _B.get("share") else psS
                    for _g in range(4):
                        st_g = spool.tile([128, 192], f32, tag="a" if _B.get("share") else "s")
                        sts.append(st_g)
                    for w in range(2):
                        for h in range(H):
                            g, i = h % 4, h // 4
                            rp = g * 32
                            j = h // 4
                            nc.tensor.matmul(
                                sts[g][w * 64 : (w + 1) * 64, i * 64 : (i + 1) * 64],
                                lhsT=qkt[rp : rp + 32, 3 + j, f0 + w * 64 : f0 + (w + 1) * 64],
                                rhs=qkt[rp : rp + 32, j, f0 + w * 64 : f0 + (w + 1) * 64],
                                start=True,
                                stop=True,
                                tile_position=(rp, w * 64),
                            )
                    er = er_pool.tile([128, 768], dt, tag="er")
                    et = et_pool.tile([128, 768], dt, tag="et")
                    for g in range(4):
                        nc.scalar.activation(er[:, g * 192 : (g + 1) * 192], sts[g][:], EXP)
                    for gg in range(4):
                        nc.vector.tensor_mul(
                            et[:, gg * 192 : (gg + 1) * 192],
                            er[:, gg * 192 : (gg + 1) * 192],
                            eb[:, gg * 192 : (gg + 1) * 192],
                        )
                    avp_t = psA.tile([128, 512], f32, tag="a")
                    avp = avp_t[:, 0 : H * 33].rearrange("p (h d) -> p h d", d=33)
                    for h in range(H):
                        g, i = h % 4, h // 4
                        ec = g * 192 + i * 64
                        for w in range(2):
                            nc.tensor.matmul(
                                avp[w * 64 : (w + 1) * 64, h, :],
                                lhsT=et[w * 64 : (w + 1) * 64, ec : ec + 64],
                                rhs=vsb[w * 64 : (w + 1) * 64, h, :],
                                start=True,
                                stop=True,
                                tile_position=(w * 64, w * 64),
                            )
                    inv = inv_pool.tile([128, H], f32, tag="inv")
                    nc.vector.reciprocal(inv[:], avp[:, :, 32])
                    avsb = av_pool.tile([128, H, 32], dt, tag="av")
                    nc.vector.tensor_mul(
                        avsb[:],
                        avp[:, :, 0:32],
                        inv[:, :, None].broadcast_to([128, H, 32]),
                    )
                    avsbs.append(avsb)

                for blk in range(NPAIR):
                    avsb = avsbs[blk]
                    avt = avt_pool.tile([128, 3, 128], dt, tag="avt")
                    av_flat = avsb[:].rearrange("p h d -> p (h d)")
                    tp = psB.tile([128, 3, 128], dt, tag="b")
                    for cc in range(3):
                        nc.tensor.transpose(
                            tp[:, cc, :], av_flat[:, cc * 128 : (cc + 1) * 128], ident[:]
                        )
                    nc.scalar.copy(avt[:], tp[:])
                    ops = psB.tile([128, DIM], f32, tag="b")
                    for cc in range(3):
                        nc.tensor.matmul(
                            ops[:],
                            lhsT=avt[:, cc, :],
                            rhs=wp[:, cc, :],
                            start=(cc == 0),
                            stop=(cc == 2),
                        )
                    nc.vector.tensor_add(osb_s[:, blk, :], ops[:], pb[:])

                nc.sync.dma_start(
                    out_d[t0 : t0 + SUPER, :].rearrange("(b p) c -> p b c", p=128),
                    osb_s[:],
                )

            def full_pass():
                for sp in range(n_super):
                    super_body(sp)

            if reps > 1:
                with tc.For_i(0, reps, 1):
                    full_pass()
            else:
                full_pass()
    nc.compile()
    return nc


def _get_nc(mode, tok_per_core, reps=1):
    key = (mode, tok_per_core, reps)
    if key not in _cache:
        _cache[key] = _build(mode, tok_per_core, reps)
    return _cache[key]


def _host_prep(x, qkv_w, proj_w, proj_b, bias_table, rel_idx, mode, n_cores):
    np_dt = ml_dtypes.bfloat16
    x = np.asarray(x, np.float32)
    qkv_w = np.asarray(qkv_w, np.float32)
    proj_w = np.asarray(proj_w, np.float32)
    proj_b = np.asarray(proj_b, np.float32)
    bias_table = np.asarray(bias_table, np.float32)
    rel_idx = np.asarray(rel_idx)

    wq = qkv_w[0:DIM] * SCALE
    wk = qkv_w[DIM : 2 * DIM]
    wv = qkv_w[2 * DIM :]
    wqkT = np.concatenate([wq, wk], 0).T.copy().astype(np_dt)  # [384, 768]
    wvT = wv.T.copy().astype(np_dt)
    wpT = proj_w.T.copy().astype(np_dt)

    bias = bias_table[rel_idx.reshape(-1)].reshape(N, N, H)  # [nq, nk, h]
    # (head, window)-packed S layout: bank g=h%4 cols [g*192,(g+1)*192)
    # holds heads g, g+4, g+8 in 64-col chunks (i=h//4); window w on
    # partition half w*64. Both halves of a chunk carry the same bias.
    ebP = np.zeros((PAIR, 768), np.float32)
    for h in range(H):
        g, i = h % 4, h // 4
        ec = g * 192 + i * 64
        bT = bias[:, :, h].T  # [nk, nq]
        for w in range(2):
            ebP[w * N : (w + 1) * N, ec : ec + N] = bT
    ebP = np.exp(ebP).astype(np_dt)  # exp(bias), [128, 768]
    pb = np.broadcast_to(proj_b, (PAIR, DIM)).copy().astype(np.float32)
    ident = np.eye(PAIR, dtype=np.float32).astype(np_dt)

    B = x.shape[0]
    bper = B // n_cores
    xs = x.reshape(B * N, DIM)
    in_maps = []
    for c in range(n_cores):
        xT = np.ascontiguousarray(
            xs[c * bper * N : (c + 1) * bper * N].T
        ).astype(np_dt)  # [384, TOK]
        in_maps.append(
            {
                "xT": xT,
                "wqkT": wqkT,
                "wvT": wvT,
                "wpT": wpT,
                "ebP": ebP,
                "pb": pb,
                "ident": ident,
            }
        )
    return in_maps


def kernel(x, qkv_w, proj_w, proj_b, bias_table, rel_idx):
    from concourse.bass_utils import run_bass_kernel_spmd

    x = np.asarray(x)
    B = x.shape[0]
    n_cores = NCORES
    tok_per_core = (B // n_cores) * N
    nc = _get_nc(MODE, tok_per_core)
    in_maps = _host_prep(x, qkv_w, proj_w, proj_b, bias_table, rel_idx, MODE, n_cores)
    res = run_bass_kernel_spmd(nc, in_maps, list(range(n_cores)))
    out = np.concatenate([r["out"] for r in res.results], 0)
    return out.reshape(B, N, DIM).astype(np.float32)
